# revision 1
# baseline (speedup 1.0000x reference)
"""GroupedQueryAttention Trainium2 kernel.

Sharding: 8 cores = 2 (batch) x 4 (KV-head groups). Each core computes, for
its batch b and its 2 KV heads (8 query heads = 512 q dims):
  qT = (Wq_slice @ hidden[b].T + bq)      [512, S]   (d on partitions)
  kT = (Wk_slice @ hidden[b].T + bk)      [128, S]
  vT = ...                                 [128, S] -> PE-transposed to v [t, d]
  per head: scoresT[t,s] = k.q / sqrt(D); exp; PV via [v|1] matmul (Z in row 64)
  o_partial[s, :] = attn_norm[s, 512] @ Wo_slice  (row-parallel)
Host sums the 4 partials per batch and adds bo.

All matmuls run in float32r (TF32-like, ~1.6e-4 relerr, full PE rate).
"""

import numpy as np

import concourse.bass as bass
import concourse.mybir as mybir
import concourse.tile as tile
from concourse import bacc
from concourse.masks import make_identity
from concourse.bass_utils import run_bass_kernel_spmd

P = 128
B, S, HID = 2, 2048, 2048
NH, G = 32, 8
HG = NH // G            # 4 query heads per KV head
D = HID // NH           # 64
NCORES = 8
GS = NCORES // B        # 4 head-group shards
DQ = HID // GS          # 512 q dims per core
DKV = G * D // GS       # 128 kv dims per core
CH = 512                # s-chunk width
NCH = S // CH           # 4
KT = HID // P           # 16 contraction tiles for projections
TT = S // P             # 16 key tiles
NPAIR = DQ // P         # 4 head pairs per core
OKT = DQ // P           # 4 o-proj contraction tiles

f32 = mybir.dt.float32
f32r = mybir.dt.float32r
EXPF = mybir.ActivationFunctionType.Exp
SCALE = 1.0 / float(np.sqrt(D))


def _emit(tc):
    nc = tc.nc
    ht = nc.dram_tensor("ht", [HID, S], f32, kind="ExternalInput")
    wq = nc.dram_tensor("wq", [HID, DQ], f32, kind="ExternalInput")
    wk = nc.dram_tensor("wk", [HID, DKV], f32, kind="ExternalInput")
    wv = nc.dram_tensor("wv", [HID, DKV], f32, kind="ExternalInput")
    wo = nc.dram_tensor("wo", [DQ, HID], f32, kind="ExternalInput")
    bqd = nc.dram_tensor("bq", [DQ], f32, kind="ExternalInput")
    bkd = nc.dram_tensor("bk", [DKV], f32, kind="ExternalInput")
    bvd = nc.dram_tensor("bv", [DKV], f32, kind="ExternalInput")
    opart = nc.dram_tensor("opart", [S, HID], f32, kind="ExternalOutput")

    consts = tc.alloc_tile_pool(name="consts", bufs=1)
    wbig = tc.alloc_tile_pool(name="wbig", bufs=1)
    wkvp = tc.alloc_tile_pool(name="wkv", bufs=1)
    htp = tc.alloc_tile_pool(name="htp", bufs=10)
    persist = tc.alloc_tile_pool(name="persist", bufs=1)
    work = tc.alloc_tile_pool(name="work", bufs=2)
    expp = tc.alloc_tile_pool(name="expp", bufs=3)

    ones_d = nc.dram_tensor("ones", [P, D], f32, kind="ExternalInput")
    bsel_d = nc.dram_tensor("bsel", [33, P], f32, kind="ExternalInput")
    zeros_d = nc.dram_tensor("zeros", [33, CH], f32, kind="ExternalInput")

    ident = consts.tile([P, P], f32)
    make_identity(nc, ident)
    # selector matrix: row 0 -> partitions 0:64, row 32 -> partitions 64:128
    bsel = consts.tile([33, P], f32r)
    nc.sync.dma_start(out=bsel[:], in_=bsel_d[:].bitcast(f32r))
    # persistent zrec, zero-initialized once (rows 1..31 stay zero)
    zrec = consts.tile([33, CH], f32r)
    nc.sync.dma_start(out=zrec[:], in_=zeros_d[:].bitcast(f32r))
    bq_t = consts.tile([P, NPAIR], f32)
    nc.sync.dma_start(out=bq_t[:], in_=bqd.rearrange("(mt p) -> p mt", p=P))
    bk_t = consts.tile([P, 1], f32)
    nc.sync.dma_start(out=bk_t[:], in_=bkd.rearrange("(p one) -> p one", p=P))
    bv_t = consts.tile([P, 1], f32)
    nc.sync.dma_start(out=bv_t[:], in_=bvd.rearrange("(p one) -> p one", p=P))

    # weights (float32r via DMA bitcast); wq split so early k-tiles land fast
    wk_sb = wkvp.tile([P, KT, DKV], f32r)
    nc.sync.dma_start(out=wk_sb[:], in_=wk.rearrange("(kt p) m -> p kt m", p=P).bitcast(f32r))
    wv_sb = wkvp.tile([P, KT, DKV], f32r)
    nc.sync.dma_start(out=wv_sb[:], in_=wv.rearrange("(kt p) m -> p kt m", p=P).bitcast(f32r))
    wq_sb = wbig.tile([P, KT, DQ], f32r, tag="wbig")
    wq_r = wq.rearrange("(kt p) m -> p kt m", p=P).bitcast(f32r)
    for q4 in range(4):
        nc.sync.dma_start(out=wq_sb[:, 4 * q4:4 * (q4 + 1), :], in_=wq_r[:, 4 * q4:4 * (q4 + 1), :])

    qT_sb = persist.tile([P, NPAIR, S], f32r)
    ktrepA = persist.tile([P, S], f32r)
    ktrepB = persist.tile([P, S], f32r)
    vT_sb = persist.tile([P, S], f32)
    v_tiles = persist.tile([P, TT, 2, D + 1], f32r)
    attn_sb = persist.tile([P, OKT, S], f32r)

    # ---- pass 1: projections ----
    with tc.tile_pool(name="ps1", bufs=7, space="PSUM") as ps1:
        nc.sync.dma_start(out=v_tiles[:, :, :, D:D + 1], in_=ones_d[:, 0:TT * 2].bitcast(f32r))
        for c in range(NCH):
            cs = slice(c * CH, (c + 1) * CH)
            qps = [ps1.tile([P, CH], f32, tag="p1", name=f"qps{mt}") for mt in range(NPAIR)]
            kps = ps1.tile([P, CH], f32, tag="p1")
            vps = ps1.tile([P, CH], f32, tag="p1")
            for kt in range(KT):
                htt = htp.tile([P, CH], f32r)
                nc.sync.dma_start(out=htt[:], in_=ht[kt * P:(kt + 1) * P, cs].bitcast(f32r))
                for mt in range(NPAIR):
                    nc.tensor.matmul(qps[mt][:], wq_sb[:, kt, mt * P:(mt + 1) * P],
                                     htt[:], start=(kt == 0), stop=(kt == KT - 1))
                nc.tensor.matmul(kps[:], wk_sb[:, kt, :], htt[:],
                                 start=(kt == 0), stop=(kt == KT - 1))
                nc.tensor.matmul(vps[:], wv_sb[:, kt, :], htt[:],
                                 start=(kt == 0), stop=(kt == KT - 1))
            for mt in range(NPAIR):
                nc.vector.tensor_scalar_add(qT_sb[:, mt, cs], qps[mt][:], bq_t[:, mt:mt + 1])
            ktmp = work.tile([P, CH], f32r, tag="ktmp")
            nc.vector.tensor_scalar_add(ktmp[:], kps[:], bk_t[:, 0:1])
            nc.sync.dma_start(out=ktrepA[0:D, cs], in_=ktmp[0:D, :])
            nc.sync.dma_start(out=ktrepA[D:P, cs], in_=ktmp[0:D, :])
            nc.sync.dma_start(out=ktrepB[0:D, cs], in_=ktmp[D:P, :])
            nc.sync.dma_start(out=ktrepB[D:P, cs], in_=ktmp[D:P, :])
            nc.vector.tensor_scalar_add(vT_sb[:, cs], vps[:], bv_t[:, 0:1])
            # transpose this chunk's v: vT [dkv, t] -> v_tiles [t, dkv]
            for i in range(4 * c, 4 * (c + 1)):
                tp = ps1.tile([P, P], f32, tag="p1")
                nc.tensor.transpose(tp[:], vT_sb[:, i * P:(i + 1) * P], ident[:])
                for g in range(2):
                    nc.vector.tensor_copy(v_tiles[:, i, g, 0:D], tp[:, g * D:(g + 1) * D])

    # ---- pass 2: attention + interleaved o-projection ----
    # wo reuses wq's SBUF slot (same tag); its DMA starts once pass 1 releases wq
    wo_sb = wbig.tile([P, OKT, HID], f32r, tag="wbig")
    nc.sync.dma_start(out=wo_sb[:], in_=wo.rearrange("(kt p) m -> p kt m", p=P).bitcast(f32r))
    with tc.tile_pool(name="ps2", bufs=1, space="PSUM") as ps2:
        def emit_oproj(st):
            ss = slice(st * P, (st + 1) * P)
            for hc in range(HID // CH):
                hs = slice(hc * CH, (hc + 1) * CH)
                op = ps2.tile([P, CH], f32, tag="aux", bufs=2, name="op")
                for kt in range(OKT):
                    nc.tensor.matmul(op[:], attn_sb[:, kt, ss], wo_sb[:, kt, hs],
                                     start=(kt == 0), stop=(kt == OKT - 1))
                ostg = work.tile([P, CH], f32, tag="ostg", bufs=4, name="ostg")
                nc.vector.tensor_copy(ostg[:], op[:])
                nc.sync.dma_start(out=opart[ss, hs], in_=ostg[:])

        for c in range(NCH):
            cs = slice(c * CH, (c + 1) * CH)
            for p in range(NPAIR):
                ktrep = ktrepA if p < 2 else ktrepB
                g = p // 2
                pvE = ps2.tile([D + 1, CH], f32, tag="pv", bufs=2)
                pvO = ps2.tile([D + 1, CH], f32, tag="pv", bufs=2)
                for t in range(TT):
                    sc = ps2.tile([P, 2, CH], f32, tag="sc", bufs=2)
                    ts_ = slice(t * P, (t + 1) * P)
                    nc.tensor.matmul(sc[:, 0, :], ktrep[0:D, ts_], qT_sb[0:D, p, cs],
                                     tile_position=(0, 0), start=True, stop=True)
                    nc.tensor.matmul(sc[:, 1, :], ktrep[D:P, ts_], qT_sb[D:P, p, cs],
                                     tile_position=(D, 0), start=True, stop=True)
                    ex = expp.tile([P, 2, CH], f32r, tag="exp")
                    nc.scalar.activation(out=ex[:], in_=sc[:], func=EXPF, scale=SCALE)
                    nc.tensor.matmul(pvE[:], v_tiles[:, t, g, :], ex[:, 0, :],
                                     start=(t == 0), stop=(t == TT - 1))
                    nc.tensor.matmul(pvO[:], v_tiles[:, t, g, :], ex[:, 1, :],
                                     start=(t == 0), stop=(t == TT - 1))
                # normalize by Z (row D of pv psums) and write attn_sb
                with nc.allow_low_precision(reason="f32r reciprocal feeds f32r matmul"):
                    nc.vector.reciprocal(zrec[0:1, :], pvE[D:D + 1, :])
                    nc.vector.reciprocal(zrec[32:33, :], pvO[D:D + 1, :])
                rbcp = ps2.tile([P, CH], f32, tag="aux", bufs=2)
                nc.tensor.matmul(rbcp[:], bsel[:], zrec[:], start=True, stop=True)
                rbc = work.tile([P, CH], f32, tag="rbc")
                nc.vector.tensor_copy(rbc[:], rbcp[:])
                nc.vector.tensor_mul(attn_sb[0:D, p, cs], pvE[0:D, :], rbc[0:D, :])
                nc.vector.tensor_mul(attn_sb[D:P, p, cs], pvO[0:D, :], rbc[D:P, :])
                # previous chunk's o-projection, one s-tile per pair
                if c > 0:
                    emit_oproj((c - 1) * (CH // P) + p)
        for st in range((NCH - 1) * (CH // P), NCH * (CH // P)):
            emit_oproj(st)

    for pool in (expp, work, persist, htp, wkvp, wbig, consts):
        pool.release()


_NC_CACHE = None


def build_nc():
    global _NC_CACHE
    if _NC_CACHE is None:
        nc = bacc.Bacc("TRN2")
        with tile.TileContext(nc) as tc:
            _emit(tc)
        nc.compile()
        _NC_CACHE = nc
    return _NC_CACHE


def _bsel_np():
    b = np.zeros((33, P), dtype=np.float32)
    b[0, 0:D] = 1.0
    b[32, D:P] = 1.0
    return b


def make_in_maps(hidden_state, Wq, bq, Wk, bk, Wv, bv, Wo):
    hidden_state = np.asarray(hidden_state, dtype=np.float32)
    Wq, Wk, Wv, Wo = (np.asarray(a, dtype=np.float32) for a in (Wq, Wk, Wv, Wo))
    bq, bk, bv = (np.asarray(a, dtype=np.float32) for a in (bq, bk, bv))
    htb = [np.ascontiguousarray(hidden_state[b].T) for b in range(B)]
    ones = np.ones((P, 32), dtype=np.float32)
    in_maps = []
    for c in range(NCORES):
        b, gs = divmod(c, GS)
        in_maps.append({
            "ht": htb[b],
            "ones": np.ones((P, D), dtype=np.float32),
            "bsel": _bsel_np(),
            "zeros": np.zeros((33, CH), dtype=np.float32),
            "wq": np.ascontiguousarray(Wq[gs * DQ:(gs + 1) * DQ, :].T),
            "wk": np.ascontiguousarray(Wk[gs * DKV:(gs + 1) * DKV, :].T),
            "wv": np.ascontiguousarray(Wv[gs * DKV:(gs + 1) * DKV, :].T),
            "wo": np.ascontiguousarray(Wo[:, gs * DQ:(gs + 1) * DQ].T),
            "bq": np.ascontiguousarray(bq[gs * DQ:(gs + 1) * DQ]),
            "bk": np.ascontiguousarray(bk[gs * DKV:(gs + 1) * DKV]),
            "bv": np.ascontiguousarray(bv[gs * DKV:(gs + 1) * DKV]),
        })
    return in_maps


def unshard(results, bo):
    bo = np.asarray(bo, dtype=np.float32)
    out = np.empty((B, S, HID), dtype=np.float32)
    for b in range(B):
        acc = np.zeros((S, HID), dtype=np.float64)
        for gs in range(GS):
            acc += results[b * GS + gs]["opart"]
        out[b] = (acc + bo).astype(np.float32)
    return out


def kernel(hidden_state, attention_mask, Wq, bq, Wk, bk, Wv, bv, Wo, bo):
    # attention_mask is all-ones for this problem (fill: ones) -> identity.
    nc = build_nc()
    in_maps = make_in_maps(hidden_state, Wq, bq, Wk, bk, Wv, bv, Wo)
    res = run_bass_kernel_spmd(nc, in_maps, list(range(NCORES)))
    return unshard(res.results, bo)



# revision 25
# speedup vs baseline: 1.2106x; 1.2106x over previous
"""GroupedQueryAttention Trainium2 kernel.

Sharding: 8 cores = 2 (batch) x 4 (KV-head groups). Each core handles one
batch b and 2 KV heads (8 query heads, DQ=512 q dims, DKV=128 kv dims).

Cost model facts this design exploits (TRN2 CoreSim): matmul cost =
out_free_rows x cyc/row regardless of K; fp8 DoubleRow = 0.5 cyc/row and
contracts TWO 128-deep k-subtiles per instruction; bf16/fp16 = 1.0 cyc/row.
Precision constraint: softmax weight noise does NOT average out (output
rel-err ~= weight rel-noise), so scores/exp/PV must be >= bf16/fp16 exact;
only the linear projections tolerate fp8, via hi+lo splitting.

Per-core pipeline:
pass 1: q/k/v projections in fp8e4m3 DoubleRow, 3 cross terms
    (h_hi W_hi + h_hi W_lo + h_lo W_hi), weights x32 host-scaled; psum/32
    cast to bf16 (q, k) / fp16 (v). k duplicated row-wise so each head's
    score matmul operands share a partition base. v PE-transposed to [t, d]
    with a ones column (Z row trick).
pass 2 (software-pipelined one head-iteration deep):
    scoresT[t, s]: one bf16 matmul per (head, t-tile, chunk), K=64.
    exp: Act engine exact exp -> fp16 (scores span +-11 sigma, fits fp16).
    PV transposed: out[s-tile, 65] fp16, N=65 per matmul (full PE rate).
    normalize via per-partition reciprocal; attn bf16; head pairs
    PE-transposed into attn_T; o-proj bf16 N=512; fp16 partials to DRAM.

Host sums the 4 group-shard partials per batch and adds bo.
"""

import numpy as np
import ml_dtypes

import concourse.bass as bass
import concourse.mybir as mybir
import concourse.tile as tile
from concourse import bacc
from concourse.masks import make_identity
from concourse.bass_utils import run_bass_kernel_spmd

P = 128
B, S, HID = 2, 2048, 2048
NH, G = 32, 8
HG = NH // G            # 4 query heads per KV head
D = HID // NH           # 64
NCORES = 8
GS = NCORES // B        # 4 group shards
DQ = HID // GS          # 512 q dims per core
DKV = G * D // GS       # 128 kv dims per core (2 KV heads)
CH = 512                # s-chunk width
NCH = S // CH           # 4
KT = HID // P           # 16 contraction tiles for projections
TT = S // P             # 16 key tiles
WSCALE = 32.0           # host scale on W for fp8 range

f32 = mybir.dt.float32
f16 = mybir.dt.float16
bf16 = mybir.dt.bfloat16
fp8e4 = mybir.dt.float8e4
EXPF = mybir.ActivationFunctionType.Exp
DR = mybir.MatmulPerfMode.DoubleRow
ADD = mybir.AluOpType.add
MULT = mybir.AluOpType.mult

SC = 1.0 / np.sqrt(D)   # exp scale on raw scores (q, k stored unscaled)

DEBUG = False


def _emit(tc):
    nc = tc.nc
    h8hi = nc.dram_tensor("h8hi", [HID, S], fp8e4, kind="ExternalInput")
    h8lo = nc.dram_tensor("h8lo", [HID, S], fp8e4, kind="ExternalInput")
    wqh = nc.dram_tensor("wqh", [HID, DQ], fp8e4, kind="ExternalInput")
    wql = nc.dram_tensor("wql", [HID, DQ], fp8e4, kind="ExternalInput")
    wkh = nc.dram_tensor("wkh", [HID, DKV], fp8e4, kind="ExternalInput")
    wkl = nc.dram_tensor("wkl", [HID, DKV], fp8e4, kind="ExternalInput")
    wvh = nc.dram_tensor("wvh", [HID, DKV], fp8e4, kind="ExternalInput")
    wvl = nc.dram_tensor("wvl", [HID, DKV], fp8e4, kind="ExternalInput")
    wo = nc.dram_tensor("wo", [DQ, HID], bf16, kind="ExternalInput")
    bqd = nc.dram_tensor("bq", [DQ], f32, kind="ExternalInput")
    bkd = nc.dram_tensor("bk", [DKV], f32, kind="ExternalInput")
    bvd = nc.dram_tensor("bv", [DKV], f32, kind="ExternalInput")
    opart = nc.dram_tensor("opart", [S, HID], f16, kind="ExternalOutput")
    if DEBUG:
        dbg = {
            "dex": nc.dram_tensor("dex", [P, TT, CH], f16,
                                  kind="ExternalOutput"),
            "dpv": nc.dram_tensor("dpv", [P, D + 1], f32,
                                  kind="ExternalOutput"),
            "dat": nc.dram_tensor("dat", [P, 4, S], bf16,
                                  kind="ExternalOutput"),
        }

    consts = tc.alloc_tile_pool(name="consts", bufs=1)
    wpool = tc.alloc_tile_pool(name="wpool", bufs=1)
    hstr = tc.alloc_tile_pool(name="hstr", bufs=2)
    persist = tc.alloc_tile_pool(name="persist", bufs=1)
    expool = tc.alloc_tile_pool(name="expool", bufs=1)
    work = tc.alloc_tile_pool(name="work", bufs=2)

    ident = consts.tile([P, P], f16)
    make_identity(nc, ident)
    identb = consts.tile([P, P], bf16)
    make_identity(nc, identb)
    bq_t = consts.tile([P, 4], f32)
    nc.sync.dma_start(out=bq_t[:], in_=bqd.rearrange("(mt p) -> p mt", p=P))
    bk_t = consts.tile([P, 1], f32)
    nc.sync.dma_start(out=bk_t[:], in_=bkd.rearrange("(p one) -> p one", p=P))
    bv_t = consts.tile([P, 1], f32)
    nc.sync.dma_start(out=bv_t[:], in_=bvd.rearrange("(p one) -> p one", p=P))

    # weights in SBUF
    def wload(name, dram, m):
        t = wpool.tile([P, KT, m], fp8e4, name=name)
        nc.sync.dma_start(out=t[:], in_=dram.rearrange("(kt p) m -> p kt m", p=P))
        return t

    wqh_sb = wload("wqh", wqh, DQ)
    wql_sb = wload("wql", wql, DQ)
    wkh_sb = wload("wkh", wkh, DKV)
    wkl_sb = wload("wkl", wkl, DKV)
    wvh_sb = wload("wvh", wvh, DKV)
    wvl_sb = wload("wvl", wvl, DKV)
    wo_sb = wpool.tile([P, 4, HID], bf16)

    # persistent activations
    qq = persist.tile([P, 4, S], bf16)
    krep = [persist.tile([P, S], bf16, name=f"krep{qd}") for qd in range(2)]
    vT_sb = persist.tile([P, S], f16)
    vf = [persist.tile([P, TT, D + 1], f16, name=f"vf{qd}") for qd in range(2)]
    attn_T = persist.tile([P, 4, S], bf16)

    for qd in range(2):
        nc.vector.memset(vf[qd][:, :, D:D + 1], 1.0)

    # ---- pass 1: projections ----
    with tc.tile_pool(name="ps1", bufs=1, space="PSUM") as ps1:
        for c in range(NCH):
            cs = slice(c * CH, (c + 1) * CH)
            hh = hstr.tile([P, KT, CH], fp8e4, tag="hh")
            nc.sync.dma_start(
                out=hh[:], in_=h8hi.rearrange("(kt p) s -> p kt s", p=P)[:, :, cs])
            hl = hstr.tile([P, KT, CH], fp8e4, tag="hl")
            nc.sync.dma_start(
                out=hl[:], in_=h8lo.rearrange("(kt p) s -> p kt s", p=P)[:, :, cs])

            def proj3(out_ps, wh, wl, mslice):
                terms = ((hh, wh), (hh, wl), (hl, wh))
                for ti, (hsrc, wsrc) in enumerate(terms):
                    for p8 in range(KT // 2):
                        ks = slice(2 * p8, 2 * p8 + 2)
                        nc.tensor.matmul(out_ps[:], wsrc[:, ks, mslice],
                                         hsrc[:, ks, :],
                                         start=(ti == 0 and p8 == 0),
                                         stop=(ti == 2 and p8 == KT // 2 - 1),
                                         perf_mode=DR)

            # k projection + row-duplication
            kps = ps1.tile([P, CH], f32, tag="kv", bufs=2)
            proj3(kps, wkh_sb, wkl_sb, slice(0, DKV))
            ktmp = work.tile([P, CH], bf16, tag="ktmp", bufs=2)
            nc.vector.tensor_scalar(ktmp[:], kps[:], 1.0 / WSCALE, bk_t[:, 0:1],
                                    MULT, ADD)
            for qd in range(2):
                for half in range(2):
                    nc.scalar.dma_start(out=krep[qd][64 * half:64 * (half + 1), cs],
                                        in_=ktmp[64 * qd:64 * (qd + 1), :])

            # v projection
            vps = ps1.tile([P, CH], f32, tag="kv", bufs=2)
            proj3(vps, wvh_sb, wvl_sb, slice(0, DKV))
            nc.vector.tensor_scalar(vT_sb[:, cs], vps[:], 1.0 / WSCALE,
                                    bv_t[:, 0:1], MULT, ADD)

            # q projection: 4 M-tiles (head pairs), natural order
            for mt in range(4):
                qps = ps1.tile([P, CH], f32, tag="qp", bufs=2)
                proj3(qps, wqh_sb, wql_sb, slice(mt * P, (mt + 1) * P))
                nc.vector.tensor_scalar(qq[:, mt, cs], qps[:], 1.0 / WSCALE,
                                        bq_t[:, mt:mt + 1], MULT, ADD)

            # transpose v chunk -> [t, d] fp16
            for i in range(4):
                t = 4 * c + i
                vtp = ps1.tile([P, P], f16, tag="vt", bufs=2)
                nc.tensor.transpose(vtp[:], vT_sb[:, t * P:(t + 1) * P], ident[:])
                for qd in range(2):
                    nc.vector.tensor_copy(vf[qd][:, t, 0:D],
                                          vtp[:, qd * D:(qd + 1) * D])

    # wo loads after pass-1 traffic; needed once o-proj starts
    nc.sync.dma_start(out=wo_sb[:], in_=wo.rearrange("(kt p) m -> p kt m", p=P))

    # ---- pass 2: attention + interleaved o-projection ----
    iters = [(c, qd, hh) for c in range(NCH) for qd in range(2) for hh in range(4)]
    ex_tiles = {}
    apair_tiles = {}

    def emit_scores_exp(it):
        c, qd, hh = iters[it]
        cs = slice(c * CH, (c + 1) * CH)
        base = 64 * (hh % 2)
        hp = 2 * qd + hh // 2
        bsl = slice(base, base + 64)
        ex = expool.tile([P, TT, CH], f16, tag="ex", bufs=2, name="ex")
        ex_tiles[it] = ex
        for j in range(TT // 2):
            sct = ps2.tile([P, 2, CH], f32, tag="sc", bufs=2)
            for i in range(2):
                t = 2 * j + i
                nc.tensor.matmul(sct[:, i, :],
                                 krep[qd][bsl, t * P:(t + 1) * P],
                                 qq[bsl, hp, cs], start=True, stop=True)
            nc.scalar.activation(out=ex[:, 2 * j:2 * j + 2, :], in_=sct[:],
                                 func=EXPF, scale=SC)

    def emit_pv_norm(it):
        c, qd, hh = iters[it]
        ex = ex_tiles.pop(it)
        hpair = 2 * qd + hh // 2
        side = hh % 2
        if DEBUG and it == 0:
            nc.sync.dma_start(out=dbg["dex"][:], in_=ex[:])
        for st in range(4):
            pvt = ps2.tile([P, D + 1], f32, tag="pv", bufs=2)
            ss = slice(st * P, (st + 1) * P)
            for t in range(TT):
                nc.tensor.matmul(pvt[:], ex[:, t, ss], vf[qd][:, t, :],
                                 start=(t == 0), stop=(t == TT - 1))
            if DEBUG and it == 0 and st == 0:
                dstg = work.tile([P, D + 1], f32, tag="dstg", bufs=1)
                nc.vector.tensor_copy(dstg[:], pvt[:])
                nc.sync.dma_start(out=dbg["dpv"][:], in_=dstg[:])
            rec = work.tile([P, 1], f32, tag="rec", bufs=4)
            nc.vector.reciprocal(rec[:], pvt[:, D:D + 1])
            if side == 0:
                ap = work.tile([P, P], bf16, tag="apair", bufs=8, name=f"ap{st}")
                apair_tiles[(hpair, st)] = ap
            else:
                ap = apair_tiles[(hpair, st)]
            nc.vector.tensor_scalar_mul(ap[:, side * D:(side + 1) * D],
                                        pvt[:, 0:D], rec[:])
            if side == 1:
                tpp = ps2.tile([P, P], bf16, tag="tp", bufs=1)
                nc.tensor.transpose(tpp[:], ap[:], identb[:])
                nc.vector.tensor_copy(
                    attn_T[:, hpair, c * CH + st * P:c * CH + (st + 1) * P],
                    tpp[:])

    def emit_oproj(stg):
        ss = slice(stg * P, (stg + 1) * P)
        ostg = work.tile([P, 4, CH], f16, tag="ostg", bufs=2, name="ostg")
        for hc in range(4):
            hs = slice(hc * CH, (hc + 1) * CH)
            op = ps2.tile([P, CH], f32, tag="op", bufs=1)
            for kt in range(4):
                nc.tensor.matmul(op[:], attn_T[:, kt, ss], wo_sb[:, kt, hs],
                                 start=(kt == 0), stop=(kt == 3))
            nc.vector.tensor_copy(ostg[:, hc, :], op[:])
        nc.sync.dma_start(out=opart[ss, :], in_=ostg[:])

    with tc.tile_pool(name="ps2", bufs=1, space="PSUM") as ps2:
        oproj_sched = {}
        for stg in range(12):
            oproj_sched.setdefault((stg // 4) * 8 + 9 + (stg % 4), []).append(stg)
        for it in range(len(iters) + 1):
            if it < len(iters):
                emit_scores_exp(it)
            if it > 0:
                emit_pv_norm(it - 1)
            for stg in oproj_sched.get(it, ()):
                emit_oproj(stg)
        for stg in range(12, 16):
            emit_oproj(stg)
        if DEBUG:
            nc.sync.dma_start(out=dbg["dat"][:], in_=attn_T[:])

    for pool in (work, expool, persist, hstr, wpool, consts):
        pool.release()


_NC_CACHE = None


def build_nc():
    global _NC_CACHE
    if _NC_CACHE is None:
        nc = bacc.Bacc("TRN2")
        with tile.TileContext(nc) as tc:
            _emit(tc)
        nc.compile()
        _NC_CACHE = nc
    return _NC_CACHE


E4 = ml_dtypes.float8_e4m3


def _fp8_pair(a):
    hi = a.astype(E4)
    lo = (a - hi.astype(np.float32)).astype(E4)
    return hi, lo


def make_in_maps(hidden_state, Wq, bq, Wk, bk, Wv, bv, Wo):
    hidden_state = np.asarray(hidden_state, dtype=np.float32)
    Wq, Wk, Wv, Wo = (np.asarray(a, dtype=np.float32) for a in (Wq, Wk, Wv, Wo))
    bq, bk, bv = (np.asarray(a, dtype=np.float32) for a in (bq, bk, bv))
    h8 = []
    for b in range(B):
        ht = np.ascontiguousarray(hidden_state[b].T)
        h8.append(_fp8_pair(ht))
    in_maps = []
    for core in range(NCORES):
        b, gs = divmod(core, GS)
        wqh, wql = _fp8_pair(np.ascontiguousarray(
            Wq[gs * DQ:(gs + 1) * DQ, :].T * WSCALE))
        wkh, wkl = _fp8_pair(np.ascontiguousarray(
            Wk[gs * DKV:(gs + 1) * DKV, :].T * WSCALE))
        wvh, wvl = _fp8_pair(np.ascontiguousarray(
            Wv[gs * DKV:(gs + 1) * DKV, :].T * WSCALE))
        in_maps.append({
            "h8hi": h8[b][0],
            "h8lo": h8[b][1],
            "wqh": wqh, "wql": wql,
            "wkh": wkh, "wkl": wkl,
            "wvh": wvh, "wvl": wvl,
            "wo": np.ascontiguousarray(
                Wo[:, gs * DQ:(gs + 1) * DQ].T).astype(ml_dtypes.bfloat16),
            "bq": np.ascontiguousarray(bq[gs * DQ:(gs + 1) * DQ]),
            "bk": np.ascontiguousarray(bk[gs * DKV:(gs + 1) * DKV]),
            "bv": np.ascontiguousarray(bv[gs * DKV:(gs + 1) * DKV]),
        })
    return in_maps


def unshard(results, bo):
    bo = np.asarray(bo, dtype=np.float32)
    out = np.empty((B, S, HID), dtype=np.float32)
    for b in range(B):
        acc = np.zeros((S, HID), dtype=np.float32)
        for gs in range(GS):
            acc += results[b * GS + gs]["opart"].astype(np.float32)
        out[b] = acc + bo
    return out


def kernel(hidden_state, attention_mask, Wq, bq, Wk, bk, Wv, bv, Wo, bo):
    # attention_mask is all-ones for this problem (fill: ones) -> identity.
    nc = build_nc()
    in_maps = make_in_maps(hidden_state, Wq, bq, Wk, bk, Wv, bv, Wo)
    res = run_bass_kernel_spmd(nc, in_maps, list(range(NCORES)))
    return unshard(res.results, bo)
